# revision 56
# baseline (speedup 1.0000x reference)
"""Local windowed MHA (lucidrains LocalAttention, window=128, look_back=1,
look_fwd=1, non-causal) on 8 TRN2 NeuronCores.

Sharding: core = batch*2 + seq_half. Each core owns 4096 tokens of one
batch element plus a 128-token halo on each side (zero-padded at true
sequence edges). Attention is local, so shards are fully independent —
no collectives.

Per-core dataflow (baseline 256.6us -> ~210us):
  - q/k projections run in fp8e4 DoubleRow mode (contraction 512 as 2
    matmuls of K=256): x and w_qkv host-interleaved to
    [ki=128, step=2, ko=2, *] with feature d = step*256 + ko*128 + ki.
    w_qkv pre-scaled by 64 (e4m3 normals start at 2^-6; raw 0.02-scale
    weights would quantize subnormally). The 64x on q,k (4096x on sim)
    folds into the exp scale.
  - v projection stays fp16 (x fp16 feature-major + w_v fp16): v errors
    pass through attention averaging at full relative strength, so fp8
    v alone costs ~2.5e-2 rel err. v is scaled by 64 too; cancels
    against w_out/64.
  - QK per (window w, kk in 0..2): one matmul per head, N=128, K=64;
    head pairs (2c, 2c+1) sit on partition halves so consecutive
    matmuls land on PE row-tiles (0,0)/(64,0) and overlap. sim psum
    [128, 8, 128] per (w, kk); pair members in different banks
    (slot = 4*(h%2) + h//2).
  - exp via one ACT per (w, kk): [128, 1024] psum -> e fp16 sbuf.
  - A@V token-major: per (w, h): 3 accumulating matmuls lhsT=e slice
    [128j, 128i], rhs=v65 [128j, 65] -> att [128i, 65]; col 64
    multiplies the vones pad indicator = exact softmax denominator.
  - normalize: reciprocal + broadcast-multiply -> attn fp16 [tok, 512].
  - attn -> feature-major via DMA sbuf->sbuf transpose on a dedicated
    queue (sync); all other DMA on the gpsimd queue.
  - out-proj fp16: lhsT = attn_fm chunk [128e, 128i], rhs = w_outT/64
    chunk [128e, 512m] -> psum [128i, 512m]; copy out, contiguous DMA.

Scheduling (the part that actually bought the speedup): the Tile
scheduler is a simulated-time list scheduler and its sim under-models
cross-engine semaphore latency, so chains pack too tight and the PE
FIFO stalls (each stall >3.4us also re-throttles the PE clock to
1.2GHz via HAM). Countermeasures:
  - software-pipelined window stages QK+exp(w) | AV+norm+transpose(w-1)
    | out-proj(w-3), with projection work-items interleaved from a
    queue so the PE always has ready work at dependency points;
  - tile_set_cur_wait paces each window slot at 4.5us in sim time
    (empirically the basin: 4.0 and 5.0 are 10-20us worse);
  - psum drains: exp + even out-copies on ACT, everything else DVE
    (putting drains that depend on late window chains onto ACT's
    strict FIFO blocks exp and collapses the pipeline);
  - PSUM banks (the binding resource): sim 2x2 + proj 2x1 + att 1 +
    out 1 = 8.
"""

import sys

sys.path.insert(0, "/opt/trn_rl_repo")

import numpy as np
import ml_dtypes

import concourse.bass as bass
import concourse.tile as tile
import concourse.mybir as mybir
from concourse import bacc
from concourse.bass_utils import run_bass_kernel_spmd

P = 128
HEADS = 8
DH = 64
W = 128  # window size
D = 512  # model dim
B = 4
SEQ = 8192
OWN = 4096  # tokens owned per core
HALO = 128
EXT = OWN + 2 * HALO  # 4352
NWIN = EXT // W  # 34 windows in shard (0 and 33 are halo)
OWIN = OWN // W  # 32 owned windows
F16 = mybir.dt.float16
F32 = mybir.dt.float32
F8E4 = mybir.dt.float8e4
DRM = mybir.MatmulPerfMode.DoubleRow
SW = 64.0  # host-side w_qkv scale
ESCALE = 0.125 / (SW * SW)
TB = 512  # proj token block

_cached = {}


def _build_program():
    nc = bacc.Bacc("TRN2", target_bir_lowering=False, debug=False, num_devices=8)

    xdr = nc.dram_tensor("xdr", [P, 2, 2, EXT], F8E4, kind="ExternalInput").ap()
    xt16 = nc.dram_tensor("xt16", [P, 4, EXT], F16, kind="ExternalInput").ap()
    wdr = nc.dram_tensor("wdr", [P, 2, 2, 2 * D], F8E4, kind="ExternalInput").ap()
    wv16 = nc.dram_tensor("wv16", [P, 4, D], F16, kind="ExternalInput").ap()
    woutT = nc.dram_tensor("woutt", [P, 4, D], F16, kind="ExternalInput").ap()
    vones = nc.dram_tensor("vones", [EXT], F16, kind="ExternalInput").ap()
    out = nc.dram_tensor("out", [OWN, D], F32, kind="ExternalOutput").ap()

    with tile.TileContext(nc) as tc:
        _emit(tc, xdr, xt16, wdr, wv16, woutT, vones, out)

    nc.compile()
    return nc


# psum slot for head h: pair (2c, 2c+1) -> slots c and 4+c (different banks)
def _slot(h):
    return 4 * (h % 2) + h // 2


def _emit(tc, xdr, xt16, wdr, wv16, woutT, vones, out):
    nc = tc.nc
    import contextlib

    ctx = contextlib.ExitStack()
    with ctx:
        const = ctx.enter_context(tc.tile_pool(name="const", bufs=1))
        # PSUM banks: sim 2x2 + proj 2x1 + att 1x1 + out 1x1 = 8
        sim_ps = ctx.enter_context(tc.tile_pool(name="sim_ps", bufs=2, space="PSUM"))
        proj_ps = ctx.enter_context(tc.tile_pool(name="proj_ps", bufs=2, space="PSUM"))
        att_ps = ctx.enter_context(tc.tile_pool(name="att_ps", bufs=1, space="PSUM"))
        out_ps = ctx.enter_context(tc.tile_pool(name="out_ps", bufs=1, space="PSUM"))
        epool = ctx.enter_context(tc.tile_pool(name="epool", bufs=10))
        spool = ctx.enter_context(tc.tile_pool(name="spool", bufs=6))
        opool = ctx.enter_context(tc.tile_pool(name="opool", bufs=4))

        # ---- persistent SBUF tensors ----
        x_sb = const.tile([P, 2, 2, EXT], F8E4)  # DR-interleaved x (q/k proj)
        xt_sb = const.tile([P, 4, EXT], F16)  # x feature-major fp16 (v proj)
        w_sb = const.tile([P, 2, 2, 2 * D], F8E4)  # DR-interleaved w_qk * 64
        wv_sb = const.tile([P, 4, D], F16)  # w_vT * 64
        wo_sb = const.tile([P, 4, D], F16)  # w_outT / 64
        k_sb = const.tile([P, 4, EXT], F16)  # k feature-major
        q_sb = const.tile([P, 4, OWN], F16)  # q feature-major (owned only)
        v_sb = const.tile([P, NWIN, HEADS, DH + 1], F16)  # v tok-major + den col
        vo_sb = const.tile([P, NWIN], F16)  # pad indicator per (tok%128, win)

        # k-section of w first so block-0 k-projection starts ASAP
        nc.sync.dma_start(w_sb[:, :, :, D : 2 * D], wdr[:, :, :, D : 2 * D])
        nc.gpsimd.dma_start(vo_sb[:], vones.rearrange("(w p) -> p w", p=P))

        nblk = (EXT + TB - 1) // TB  # 9 (last block 320)

        def dr_proj(ps_col, eoff, ecols, toff, tcols):
            for s in range(2):
                nc.tensor.matmul(
                    ps_col,
                    lhsT=w_sb[:, s, :, eoff : eoff + ecols],
                    rhs=x_sb[:, s, :, toff : toff + tcols],
                    start=(s == 0),
                    stop=(s == 1),
                    perf_mode=DRM,
                )

        def kcast(dst, src):
            nc.vector.tensor_copy(dst, src)

        def q_chunk(qb, ec):
            qt0 = qb * TB
            qtb = min(TB, OWN - qt0)
            ps = proj_ps.tile([P, TB], F32, tag="proj", name="psq")
            dr_proj(ps[:, :qtb], ec * P, P, HALO + qt0, qtb)
            kcast(q_sb[:, ec, qt0 : qt0 + qtb], ps[:, :qtb])

        def v_window(wv):
            ps = proj_ps.tile([P, TB], F32, tag="proj", name="psv")
            for s in range(4):
                nc.tensor.matmul(
                    ps[:],
                    lhsT=xt_sb[:, s, wv * W : (wv + 1) * W],
                    rhs=wv_sb[:, s, :],
                    start=(s == 0),
                    stop=(s == 3),
                )
            nc.vector.tensor_copy(
                v_sb[:, wv, :, 0:DH], ps.rearrange("p (h e) -> p h e", h=HEADS)
            )
            if 1 <= wv <= NWIN - 2:
                nc.vector.memset(v_sb[:, wv, :, DH : DH + 1], 1.0)
            else:
                nc.vector.tensor_copy(
                    v_sb[:, wv, :, DH : DH + 1],
                    vo_sb[:, wv : wv + 1, None].to_broadcast((P, HEADS, 1)),
                )

        e_store = {}
        fm_store = {}

        def emit_qk(w):
            # owned window w = shard window w+1; attends shard kw w..w+2
            tiles = []
            for kk in range(3):
                sim = sim_ps.tile([P, HEADS, W], F32, tag="sim", name="sim")
                kwv = w + kk
                for c in range(4):
                    for hh in range(2):
                        h = 2 * c + hh
                        nc.tensor.matmul(
                            sim[:, _slot(h), :],
                            lhsT=k_sb[
                                hh * DH : (hh + 1) * DH, c, kwv * W : (kwv + 1) * W
                            ],
                            rhs=q_sb[hh * DH : (hh + 1) * DH, c, w * W : (w + 1) * W],
                            start=True,
                            stop=True,
                        )
                e = epool.tile([P, HEADS, W], F16, tag="e", name="e")
                nc.scalar.activation(
                    e[:], sim[:], mybir.ActivationFunctionType.Exp, scale=ESCALE
                )
                tiles.append(e)
            e_store[w] = tiles

        def emit_av(w, half):
            if half == 0:
                attn = spool.tile([P, HEADS, DH], F16, tag="attn", name="attn")
                attn_fm = spool.tile([P, 4, W], F16, tag="attn_fm", name="attn_fm")
                e_store[w] = (e_store[w], attn, attn_fm)
            e_tiles, attn, attn_fm = e_store[w]
            if True:
                att = att_ps.tile([P, 4, DH + 1], F32, tag="att", name="att")
                for hq in range(4):
                    h = 4 * half + hq
                    for kk in range(3):
                        nc.tensor.matmul(
                            att[:, hq, :],
                            lhsT=e_tiles[kk][:, _slot(h), :],
                            rhs=v_sb[:, w + kk, h, :],
                            start=(kk == 0),
                            stop=(kk == 2),
                        )
                recip = spool.tile([P, 4, 1], F32, tag="recip", name="recip")
                nc.vector.reciprocal(recip[:], att[:, :, DH : DH + 1])
                nc.vector.tensor_tensor(
                    attn[:, 4 * half : 4 * half + 4, :],
                    att[:, :, 0:DH],
                    recip[:, :, 0:1].to_broadcast((P, 4, DH)),
                    mybir.AluOpType.mult,
                )
            if half == 1:
                nc.sync.dma_start_transpose(
                    attn_fm[:], attn.rearrange("p h d -> p (h d)")
                )
                fm_store[w] = attn_fm
                e_store.pop(w)

        def emit_out(w):
            attn_fm = fm_store.pop(w)
            out_psum = out_ps.tile([P, D], F32, tag="outp", name="outp")
            for c in range(4):
                nc.tensor.matmul(
                    out_psum[:],
                    lhsT=attn_fm[:, c, :],
                    rhs=wo_sb[:, c, :],
                    start=(c == 0),
                    stop=(c == 3),
                )
            out_sb = opool.tile([P, D], F32, tag="osb", name="osb")
            nc.scalar.copy(out_sb[:], out_psum[:])
            nc.gpsimd.dma_start(out[w * W : (w + 1) * W, :], out_sb[:])

        def emit_window(w):
            # software-pipelined: QK+exp(w) | AV+norm+transpose(w-1) | out(w-3)
            emit_qk(w)
            if w >= 1:
                emit_av(w - 1)
            if w >= 3:
                emit_out(w - 3)

        qdone = [0, 0]  # cols, blocks
        vdone = [0]
        wcur = [0]
        from collections import deque

        proj_q = deque()
        popped = {"k": 0, "v": 0, "q": 0}  # cols / windows / cols emitted

        def pop_proj(n=1):
            for _ in range(n):
                if proj_q:
                    kind, amt, fn = proj_q.popleft()
                    fn()
                    popped[kind] = amt

        def slot_ready(t):
            return (
                popped["q"] >= (t + 1) * W
                and popped["v"] >= t + 3
                and popped["k"] >= (t + 3) * W
            )

        def emit_slot(t):
            # force prerequisites, then interleave leftover projection work
            # between pipeline stages so the PE queue always has ready work
            tc.tile_set_cur_wait(0.02 + t * 0.0045)
            while not slot_ready(t):
                pop_proj(1)
            emit_qk(t)
            pop_proj(1)
            tc.tile_set_cur_wait(0.0215 + t * 0.0045)
            if t >= 1:
                emit_av(t - 1, 0)
            if t >= 3:
                emit_out(t - 3)
            pop_proj(1)
            tc.tile_set_cur_wait(0.023 + t * 0.0045)
            if t >= 1:
                emit_av(t - 1, 1)
            pop_proj(1)

        # all input DMAs up front (per-block slices keep deps fine-grained);
        # they stream on the gpsimd queue well ahead of compute
        for blk in range(nblk):
            t0 = blk * TB
            L = min(t0 + TB, EXT)
            nc.gpsimd.dma_start(x_sb[:, :, :, t0:L], xdr[:, :, :, t0:L])
            nc.gpsimd.dma_start(xt_sb[:, :, t0:L], xt16[:, :, t0:L])
            if blk == 0:
                nc.gpsimd.dma_start(wv_sb[:], wv16)
                nc.gpsimd.dma_start(w_sb[:, :, :, :D], wdr[:, :, :, :D])
                nc.gpsimd.dma_start(wo_sb[:], woutT)

        for blk in range(nblk):
            t0 = blk * TB
            tb = min(TB, EXT - t0)
            L = t0 + tb

            def k_chunk(ec, t0=t0, tb=tb, L=L):
                ps = proj_ps.tile([P, TB], F32, tag="proj", name="psk")
                dr_proj(ps[:, :tb], D + ec * P, P, t0, tb)
                kcast(k_sb[:, ec, t0:L], ps[:, :tb])

            kprev = blk * TB
            for ec in range(4):
                proj_q.append(("k", L if ec == 3 else kprev,
                               lambda ec=ec, kc=k_chunk: kc(ec)))
            while (vdone[0] + 1) * W <= L:
                proj_q.append(("v", vdone[0] + 1, lambda wv=vdone[0]: v_window(wv)))
                vdone[0] += 1
            while qdone[1] < (OWN + TB - 1) // TB:
                qt0 = qdone[1] * TB
                qtb = min(TB, OWN - qt0)
                if HALO + qt0 + qtb > L:
                    break
                for ec in range(4):
                    proj_q.append(("q", (qt0 + qtb) if ec == 3 else qt0,
                                   lambda qb=qdone[1], ec=ec: q_chunk(qb, ec)))
                qdone[0] = qt0 + qtb
                qdone[1] += 1

            while (
                wcur[0] < OWIN
                and (wcur[0] + 1) * W <= qdone[0]
                and (wcur[0] + 3) * W <= L
            ):
                emit_slot(wcur[0])
                wcur[0] += 1
        pop_proj(len(proj_q))
        emit_av(OWIN - 1, 0)
        emit_av(OWIN - 1, 1)
        for w in (OWIN - 3, OWIN - 2, OWIN - 1):
            emit_out(w)


def _get_program():

    if "nc" not in _cached:
        _cached["nc"] = _build_program()
    return _cached["nc"]


def _dr_interleave(mat):
    """[rows=512, cols] -> [128, 2, 2, cols] with row = s*256 + ko*128 + ki."""
    r, c = mat.shape
    assert r == D
    return np.ascontiguousarray(mat.reshape(2, 2, P, c).transpose(2, 0, 1, 3))


def _make_in_maps(x, w_qkv, w_out):
    f16 = np.float16
    f8 = ml_dtypes.float8_e4m3
    wqkvT = np.ascontiguousarray(np.asarray(w_qkv, np.float32).T) * SW  # [512, 1536]
    wdr = _dr_interleave(wqkvT[:, : 2 * D]).astype(f8)
    wv16 = np.ascontiguousarray(
        wqkvT[:, 2 * D :].reshape(4, P, D).transpose(1, 0, 2)
    ).astype(f16)
    woutT = (np.ascontiguousarray(np.asarray(w_out, np.float32).T) / SW).astype(f16)
    woutT = np.ascontiguousarray(woutT.reshape(4, P, D).transpose(1, 0, 2))
    x = np.asarray(x, np.float32)
    in_maps = []
    for core in range(8):
        b, half = core // 2, core % 2
        s = half * OWN
        lo, hi = s - HALO, s + OWN + HALO
        xs = np.zeros((EXT, D), np.float32)
        src_lo, src_hi = max(lo, 0), min(hi, SEQ)
        xs[src_lo - lo : src_hi - lo] = x[b, src_lo:src_hi]
        xsT = np.ascontiguousarray(xs.T)  # [512, EXT]
        xdr = _dr_interleave(xsT).astype(f8)
        xt = np.ascontiguousarray(xsT.reshape(4, P, EXT).transpose(1, 0, 2)).astype(
            f16
        )
        vo = np.zeros(EXT, np.float32)
        vo[src_lo - lo : src_hi - lo] = 1.0
        in_maps.append(
            {
                "xdr": xdr,
                "xt16": xt,
                "wdr": wdr,
                "wv16": wv16,
                "woutt": woutT,
                "vones": vo.astype(f16),
            }
        )
    return in_maps


def run(x, w_qkv, w_out, trace=False, **spmd_kwargs):
    nc = _get_program()
    in_maps = _make_in_maps(x, w_qkv, w_out)
    res = run_bass_kernel_spmd(nc, in_maps, list(range(8)), trace=trace, **spmd_kwargs)
    out = np.empty((B, SEQ, D), np.float32)
    for core in range(8):
        b, half = core // 2, core % 2
        out[b, half * OWN : (half + 1) * OWN] = res.results[core]["out"]
    return out, res


def kernel(x, w_qkv, w_out):
    out, _ = run(x, w_qkv, w_out)
    return out


# revision 57
# speedup vs baseline: 1.1126x; 1.1126x over previous
"""Local windowed MHA (lucidrains LocalAttention, window=128, look_back=1,
look_fwd=1, non-causal) on 8 TRN2 NeuronCores.

Sharding: core = batch*2 + seq_half. Each core owns 4096 tokens of one
batch element plus a 128-token halo on each side (zero-padded at true
sequence edges). Attention is local, so shards are fully independent —
no collectives.

Per-core dataflow (baseline 256.6us -> ~210us):
  - q/k projections run in fp8e4 DoubleRow mode (contraction 512 as 2
    matmuls of K=256): x and w_qkv host-interleaved to
    [ki=128, step=2, ko=2, *] with feature d = step*256 + ko*128 + ki.
    w_qkv pre-scaled by 64 (e4m3 normals start at 2^-6; raw 0.02-scale
    weights would quantize subnormally). The 64x on q,k (4096x on sim)
    folds into the exp scale.
  - v projection stays fp16 (x fp16 feature-major + w_v fp16): v errors
    pass through attention averaging at full relative strength, so fp8
    v alone costs ~2.5e-2 rel err. v is scaled by 64 too; cancels
    against w_out/64.
  - QK per (window w, kk in 0..2): one matmul per head, N=128, K=64;
    head pairs (2c, 2c+1) sit on partition halves so consecutive
    matmuls land on PE row-tiles (0,0)/(64,0) and overlap. sim psum
    [128, 8, 128] per (w, kk); pair members in different banks
    (slot = 4*(h%2) + h//2).
  - exp via one ACT per (w, kk): [128, 1024] psum -> e fp16 sbuf.
  - A@V token-major: per (w, h): 3 accumulating matmuls lhsT=e slice
    [128j, 128i], rhs=v65 [128j, 65] -> att [128i, 65]; col 64
    multiplies the vones pad indicator = exact softmax denominator.
  - normalize: reciprocal + broadcast-multiply -> attn fp16 [tok, 512].
  - attn -> feature-major via DMA sbuf->sbuf transpose on a dedicated
    queue (sync); all other DMA on the gpsimd queue.
  - out-proj fp16: lhsT = attn_fm chunk [128e, 128i], rhs = w_outT/64
    chunk [128e, 512m] -> psum [128i, 512m]; copy out, contiguous DMA.

Scheduling (the part that actually bought the speedup): the Tile
scheduler is a simulated-time list scheduler and its sim under-models
cross-engine semaphore latency, so chains pack too tight and the PE
FIFO stalls (each stall >3.4us also re-throttles the PE clock to
1.2GHz via HAM). Countermeasures:
  - software-pipelined window stages QK+exp(w) | AV+norm+transpose(w-1)
    | out-proj(w-3), with projection work-items interleaved from a
    queue so the PE always has ready work at dependency points;
  - tile_set_cur_wait paces each window slot at 4.5us in sim time
    (empirically the basin: 4.0 and 5.0 are 10-20us worse);
  - psum drains: exp + even out-copies on ACT, everything else DVE
    (putting drains that depend on late window chains onto ACT's
    strict FIFO blocks exp and collapses the pipeline);
  - PSUM banks (the binding resource): sim 2x2 + proj 2x1 + att 1 +
    out 1 = 8.
"""

import sys

sys.path.insert(0, "/opt/trn_rl_repo")

import numpy as np
import ml_dtypes

import concourse.bass as bass
import concourse.tile as tile
import concourse.mybir as mybir
from concourse import bacc
from concourse.bass_utils import run_bass_kernel_spmd

P = 128
HEADS = 8
DH = 64
W = 128  # window size
D = 512  # model dim
B = 4
SEQ = 8192
OWN = 4096  # tokens owned per core
HALO = 128
EXT = OWN + 2 * HALO  # 4352
NWIN = EXT // W  # 34 windows in shard (0 and 33 are halo)
OWIN = OWN // W  # 32 owned windows
F16 = mybir.dt.float16
F32 = mybir.dt.float32
F8E4 = mybir.dt.float8e4
DRM = mybir.MatmulPerfMode.DoubleRow
SW = 64.0  # host-side w_qkv scale
ESCALE = 0.125 / (SW * SW)
TB = 512  # proj token block

_cached = {}


def _build_program():
    nc = bacc.Bacc("TRN2", target_bir_lowering=False, debug=False, num_devices=8)

    xdr = nc.dram_tensor("xdr", [P, 2, 2, EXT], F8E4, kind="ExternalInput").ap()
    xt16 = nc.dram_tensor("xt16", [P, 4, EXT], F16, kind="ExternalInput").ap()
    wdr = nc.dram_tensor("wdr", [P, 2, 2, 2 * D], F8E4, kind="ExternalInput").ap()
    wv16 = nc.dram_tensor("wv16", [P, 4, D], F16, kind="ExternalInput").ap()
    woutT = nc.dram_tensor("woutt", [P, 4, D], F16, kind="ExternalInput").ap()
    vones = nc.dram_tensor("vones", [EXT], F16, kind="ExternalInput").ap()
    out = nc.dram_tensor("out", [OWN, D], F32, kind="ExternalOutput").ap()

    with tile.TileContext(nc) as tc:
        _emit(tc, xdr, xt16, wdr, wv16, woutT, vones, out)

    nc.compile()
    return nc


# psum slot for head h: pair (2c, 2c+1) -> slots c and 4+c (different banks)
def _slot(h):
    return 4 * (h % 2) + h // 2


def _emit(tc, xdr, xt16, wdr, wv16, woutT, vones, out):
    nc = tc.nc
    import contextlib

    ctx = contextlib.ExitStack()
    with ctx:
        const = ctx.enter_context(tc.tile_pool(name="const", bufs=1))
        # PSUM banks: sim 2x2 + proj 2x1 + att 1x1 + out 1x1 = 8
        sim_ps = ctx.enter_context(tc.tile_pool(name="sim_ps", bufs=2, space="PSUM"))
        proj_ps = ctx.enter_context(tc.tile_pool(name="proj_ps", bufs=2, space="PSUM"))
        att_ps = ctx.enter_context(tc.tile_pool(name="att_ps", bufs=1, space="PSUM"))
        out_ps = ctx.enter_context(tc.tile_pool(name="out_ps", bufs=1, space="PSUM"))
        epool = ctx.enter_context(tc.tile_pool(name="epool", bufs=10))
        spool = ctx.enter_context(tc.tile_pool(name="spool", bufs=6))
        opool = ctx.enter_context(tc.tile_pool(name="opool", bufs=4))

        # ---- persistent SBUF tensors ----
        x_sb = const.tile([P, 2, 2, EXT], F8E4)  # DR-interleaved x (q/k proj)
        xt_sb = const.tile([P, 4, EXT], F16)  # x feature-major fp16 (v proj)
        w_sb = const.tile([P, 2, 2, 2 * D], F8E4)  # DR-interleaved w_qk * 64
        wv_sb = const.tile([P, 4, D], F16)  # w_vT * 64
        wo_sb = const.tile([P, 4, D], F16)  # w_outT / 64
        k_sb = const.tile([P, 4, EXT], F16)  # k feature-major
        q_sb = const.tile([P, 4, OWN], F16)  # q feature-major (owned only)
        v_sb = const.tile([P, NWIN, HEADS, DH + 1], F16)  # v tok-major + den col
        vo_sb = const.tile([P, NWIN], F16)  # pad indicator per (tok%128, win)

        # k-section of w first so block-0 k-projection starts ASAP
        nc.sync.dma_start(w_sb[:, :, :, D : 2 * D], wdr[:, :, :, D : 2 * D])
        nc.gpsimd.dma_start(vo_sb[:], vones.rearrange("(w p) -> p w", p=P))

        nblk = (EXT + TB - 1) // TB  # 9 (last block 320)

        def dr_proj(ps_col, eoff, ecols, toff, tcols):
            for s in range(2):
                nc.tensor.matmul(
                    ps_col,
                    lhsT=w_sb[:, s, :, eoff : eoff + ecols],
                    rhs=x_sb[:, s, :, toff : toff + tcols],
                    start=(s == 0),
                    stop=(s == 1),
                    perf_mode=DRM,
                )

        def kcast(dst, src):
            nc.vector.tensor_copy(dst, src)

        def q_chunk(qb, ec):
            qt0 = qb * TB
            qtb = min(TB, OWN - qt0)
            ps = proj_ps.tile([P, TB], F32, tag="proj", name="psq")
            dr_proj(ps[:, :qtb], ec * P, P, HALO + qt0, qtb)
            kcast(q_sb[:, ec, qt0 : qt0 + qtb], ps[:, :qtb])

        def v_window(wv):
            ps = proj_ps.tile([P, TB], F32, tag="proj", name="psv")
            for s in range(4):
                nc.tensor.matmul(
                    ps[:],
                    lhsT=xt_sb[:, s, wv * W : (wv + 1) * W],
                    rhs=wv_sb[:, s, :],
                    start=(s == 0),
                    stop=(s == 3),
                )
            nc.vector.tensor_copy(
                v_sb[:, wv, :, 0:DH], ps.rearrange("p (h e) -> p h e", h=HEADS)
            )
            if 1 <= wv <= NWIN - 2:
                nc.vector.memset(v_sb[:, wv, :, DH : DH + 1], 1.0)
            else:
                nc.vector.tensor_copy(
                    v_sb[:, wv, :, DH : DH + 1],
                    vo_sb[:, wv : wv + 1, None].to_broadcast((P, HEADS, 1)),
                )

        e_store = {}
        fm_store = {}

        def emit_qk(w):
            # owned window w = shard window w+1; attends shard kw w..w+2
            tiles = []
            for kk in range(3):
                sim = sim_ps.tile([P, HEADS, W], F32, tag="sim", name="sim")
                kwv = w + kk
                for c in range(4):
                    for hh in range(2):
                        h = 2 * c + hh
                        nc.tensor.matmul(
                            sim[:, _slot(h), :],
                            lhsT=k_sb[
                                hh * DH : (hh + 1) * DH, c, kwv * W : (kwv + 1) * W
                            ],
                            rhs=q_sb[hh * DH : (hh + 1) * DH, c, w * W : (w + 1) * W],
                            start=True,
                            stop=True,
                        )
                e = epool.tile([P, HEADS, W], F16, tag="e", name="e")
                nc.scalar.activation(
                    e[:], sim[:], mybir.ActivationFunctionType.Exp, scale=ESCALE
                )
                tiles.append(e)
            e_store[w] = tiles

        def emit_av(w, half):
            if half == 0:
                attn = spool.tile([P, HEADS, DH], F16, tag="attn", name="attn")
                attn_fm = spool.tile([P, 4, W], F16, tag="attn_fm", name="attn_fm")
                e_store[w] = (e_store[w], attn, attn_fm)
            e_tiles, attn, attn_fm = e_store[w]
            if True:
                att = att_ps.tile([P, 4, DH + 1], F32, tag="att", name="att")
                for hq in range(4):
                    h = 4 * half + hq
                    for kk in range(3):
                        nc.tensor.matmul(
                            att[:, hq, :],
                            lhsT=e_tiles[kk][:, _slot(h), :],
                            rhs=v_sb[:, w + kk, h, :],
                            start=(kk == 0),
                            stop=(kk == 2),
                        )
                recip = spool.tile([P, 4, 1], F32, tag="recip", name="recip")
                nc.vector.reciprocal(recip[:], att[:, :, DH : DH + 1])
                nc.vector.tensor_tensor(
                    attn[:, 4 * half : 4 * half + 4, :],
                    att[:, :, 0:DH],
                    recip[:, :, 0:1].to_broadcast((P, 4, DH)),
                    mybir.AluOpType.mult,
                )
            if half == 1:
                nc.sync.dma_start_transpose(
                    attn_fm[:], attn.rearrange("p h d -> p (h d)")
                )
                fm_store[w] = attn_fm
                e_store.pop(w)

        def emit_out(w):
            attn_fm = fm_store.pop(w)
            out_psum = out_ps.tile([P, D], F32, tag="outp", name="outp")
            for c in range(4):
                nc.tensor.matmul(
                    out_psum[:],
                    lhsT=attn_fm[:, c, :],
                    rhs=wo_sb[:, c, :],
                    start=(c == 0),
                    stop=(c == 3),
                )
            out_sb = opool.tile([P, D], F32, tag="osb", name="osb")
            if w % 2 == 0:
                nc.scalar.copy(out_sb[:], out_psum[:])
            else:
                nc.vector.tensor_copy(out_sb[:], out_psum[:])
            nc.gpsimd.dma_start(out[w * W : (w + 1) * W, :], out_sb[:])

        def emit_window(w):
            # software-pipelined: QK+exp(w) | AV+norm+transpose(w-1) | out(w-3)
            emit_qk(w)
            if w >= 1:
                emit_av(w - 1)
            if w >= 3:
                emit_out(w - 3)

        qdone = [0, 0]  # cols, blocks
        vdone = [0]
        wcur = [0]
        from collections import deque

        proj_q = deque()
        popped = {"k": 0, "v": 0, "q": 0}  # cols / windows / cols emitted

        def pop_proj(n=1):
            for _ in range(n):
                if proj_q:
                    kind, amt, fn = proj_q.popleft()
                    fn()
                    popped[kind] = amt

        def slot_ready(t):
            return (
                popped["q"] >= (t + 1) * W
                and popped["v"] >= t + 3
                and popped["k"] >= (t + 3) * W
            )

        def emit_slot(t):
            # force prerequisites, then interleave leftover projection work
            # between pipeline stages so the PE queue always has ready work
            tc.tile_set_cur_wait(0.02 + t * 0.0045)
            while not slot_ready(t):
                pop_proj(1)
            emit_qk(t)
            pop_proj(1)
            tc.tile_set_cur_wait(0.0215 + t * 0.0045)
            if t >= 1:
                emit_av(t - 1, 0)
            if t >= 3:
                emit_out(t - 3)
            pop_proj(1)
            tc.tile_set_cur_wait(0.023 + t * 0.0045)
            if t >= 1:
                emit_av(t - 1, 1)
            pop_proj(1)

        # all input DMAs up front (per-block slices keep deps fine-grained);
        # they stream on the gpsimd queue well ahead of compute
        for blk in range(nblk):
            t0 = blk * TB
            L = min(t0 + TB, EXT)
            nc.gpsimd.dma_start(x_sb[:, :, :, t0:L], xdr[:, :, :, t0:L])
            nc.gpsimd.dma_start(xt_sb[:, :, t0:L], xt16[:, :, t0:L])
            if blk == 0:
                nc.gpsimd.dma_start(wv_sb[:], wv16)
                nc.gpsimd.dma_start(w_sb[:, :, :, :D], wdr[:, :, :, :D])
                nc.gpsimd.dma_start(wo_sb[:], woutT)

        for blk in range(nblk):
            t0 = blk * TB
            tb = min(TB, EXT - t0)
            L = t0 + tb

            def k_chunk(ec, t0=t0, tb=tb, L=L):
                ps = proj_ps.tile([P, TB], F32, tag="proj", name="psk")
                dr_proj(ps[:, :tb], D + ec * P, P, t0, tb)
                kcast(k_sb[:, ec, t0:L], ps[:, :tb])

            kprev = blk * TB
            for ec in range(4):
                proj_q.append(("k", L if ec == 3 else kprev,
                               lambda ec=ec, kc=k_chunk: kc(ec)))
            while (vdone[0] + 1) * W <= L:
                proj_q.append(("v", vdone[0] + 1, lambda wv=vdone[0]: v_window(wv)))
                vdone[0] += 1
            while qdone[1] < (OWN + TB - 1) // TB:
                qt0 = qdone[1] * TB
                qtb = min(TB, OWN - qt0)
                if HALO + qt0 + qtb > L:
                    break
                for ec in range(4):
                    proj_q.append(("q", (qt0 + qtb) if ec == 3 else qt0,
                                   lambda qb=qdone[1], ec=ec: q_chunk(qb, ec)))
                qdone[0] = qt0 + qtb
                qdone[1] += 1

            while (
                wcur[0] < OWIN
                and (wcur[0] + 1) * W <= qdone[0]
                and (wcur[0] + 3) * W <= L
            ):
                emit_slot(wcur[0])
                wcur[0] += 1
        pop_proj(len(proj_q))
        emit_av(OWIN - 1, 0)
        emit_av(OWIN - 1, 1)
        for w in (OWIN - 3, OWIN - 2, OWIN - 1):
            emit_out(w)


def _get_program():

    if "nc" not in _cached:
        _cached["nc"] = _build_program()
    return _cached["nc"]


def _dr_interleave(mat):
    """[rows=512, cols] -> [128, 2, 2, cols] with row = s*256 + ko*128 + ki."""
    r, c = mat.shape
    assert r == D
    return np.ascontiguousarray(mat.reshape(2, 2, P, c).transpose(2, 0, 1, 3))


def _make_in_maps(x, w_qkv, w_out):
    f16 = np.float16
    f8 = ml_dtypes.float8_e4m3
    wqkvT = np.ascontiguousarray(np.asarray(w_qkv, np.float32).T) * SW  # [512, 1536]
    wdr = _dr_interleave(wqkvT[:, : 2 * D]).astype(f8)
    wv16 = np.ascontiguousarray(
        wqkvT[:, 2 * D :].reshape(4, P, D).transpose(1, 0, 2)
    ).astype(f16)
    woutT = (np.ascontiguousarray(np.asarray(w_out, np.float32).T) / SW).astype(f16)
    woutT = np.ascontiguousarray(woutT.reshape(4, P, D).transpose(1, 0, 2))
    x = np.asarray(x, np.float32)
    in_maps = []
    for core in range(8):
        b, half = core // 2, core % 2
        s = half * OWN
        lo, hi = s - HALO, s + OWN + HALO
        xs = np.zeros((EXT, D), np.float32)
        src_lo, src_hi = max(lo, 0), min(hi, SEQ)
        xs[src_lo - lo : src_hi - lo] = x[b, src_lo:src_hi]
        xsT = np.ascontiguousarray(xs.T)  # [512, EXT]
        xdr = _dr_interleave(xsT).astype(f8)
        xt = np.ascontiguousarray(xsT.reshape(4, P, EXT).transpose(1, 0, 2)).astype(
            f16
        )
        vo = np.zeros(EXT, np.float32)
        vo[src_lo - lo : src_hi - lo] = 1.0
        in_maps.append(
            {
                "xdr": xdr,
                "xt16": xt,
                "wdr": wdr,
                "wv16": wv16,
                "woutt": woutT,
                "vones": vo.astype(f16),
            }
        )
    return in_maps


def run(x, w_qkv, w_out, trace=False, **spmd_kwargs):
    nc = _get_program()
    in_maps = _make_in_maps(x, w_qkv, w_out)
    res = run_bass_kernel_spmd(nc, in_maps, list(range(8)), trace=trace, **spmd_kwargs)
    out = np.empty((B, SEQ, D), np.float32)
    for core in range(8):
        b, half = core // 2, core % 2
        out[b, half * OWN : (half + 1) * OWN] = res.results[core]["out"]
    return out, res


def kernel(x, w_qkv, w_out):
    out, _ = run(x, w_qkv, w_out)
    return out
